# revision 43
# baseline (speedup 1.0000x reference)
"""LIF spiking-neuron (soft reset) Bass kernel for Trainium2, 8-core SPMD.

Input  x: [B=32, C=128, T=16, H=32, W=32] f32
Output s: same shape, spikes in {0, 1}.

Recurrence per element over T:
    m' = z * 0.75 + x_t              (integrate; z = post-reset membrane)
    s_t = (m' > 0.5)                 (spike)
    z   = m' - 0.5 * s_t             (soft reset)

Engine assignment (the point of this design): on TRN2, DVE and GpSimd
arbitrate an exclusive shared SBUF port pair, so GpSimd buys nothing while
DVE runs 2-src ops back-to-back; and every op here except the spike compare
is unavailable or slow elsewhere. The schedule keeps only two ops per
element on the DVE and farms the reset out to the private-port engines:

    DVE : m'_g(SBUF) = stt(z_g(PSUM) * beta + x_t,g)         ~1.2 us
    DVE : s_g(SBUF)  = (m'_g > 0.5) -> fp8e4 {0,1}           ~0.7 us
    ACT : copy m'_g -> z_g (PSUM overwrite; private ports)   ~1.1 us
    PE  : z_g += (-0.5 I_fp8) @ s_g  (matmul accumulate onto the ACT-written
          value; start=False adds unconditionally once the bank's per-element
          has_written bits are set - the t=0 zero-weight pass does that)
    DMA : s shipped as raw fp8 bytes (1B/elem); host maps to f32 {0,1}

All f32 roundings match the reference order bit-for-bit (the fp8 matmul
products are 1.0*-0.5 or 0.0, and m' - 0.5*s is exactly representable), so
rel err is 0. The spike tile doubles as the PE operand and the DMA output.

x loads and s stores are coalesced two timesteps per DMA: the DRAM layout
[row, t, f] makes a 2-step slice 8KB-contiguous per row, so transfers are
half as many with double the line length. The first load covers only t=0 to
keep the startup ramp short.

Sharding: batch dim split across 8 cores (4 per core); per core the shard is
[512 (b*c) rows, 16 t, 1024 hw], rows mapped to SBUF partitions in 4 groups
of 128. z lives in PSUM: 4 groups x 4KB = all 8 banks.
"""

import numpy as np

import concourse.bacc as bacc
import concourse.mybir as mybir
import concourse.tile as tile
from concourse.bass_utils import run_bass_kernel_spmd

B, C, T, H, W = 32, 128, 16, 32, 32
NCORES = 8
B_PER = B // NCORES          # 4
ROWS = B_PER * C             # 512
HW = H * W                   # 1024
P = 128
NG = ROWS // P               # 4 partition groups
BETA = 0.75
THRESH = 0.5

F32 = mybir.dt.float32
FP8 = mybir.dt.float8e4
ALU = mybir.AluOpType

_nc_cache = None


def _build():
    import ml_dtypes

    nc = bacc.Bacc(
        "TRN2",
        target_bir_lowering=False,
        debug=False,
        enable_asserts=False,
        num_devices=NCORES,
    )
    x_d = nc.dram_tensor("x", [ROWS, T, HW], F32, kind="ExternalInput").ap()
    s_d = nc.dram_tensor("s", [ROWS, T, HW], FP8, kind="ExternalOutput").ap()

    x_v = x_d.rearrange("(g p) t f -> g p t f", p=P)
    s_v = s_d.rearrange("(g p) t f -> g p t f", p=P)

    wn8_d = nc.inline_tensor(
        (-THRESH * np.eye(P)).astype(ml_dtypes.float8_e4m3fn), name="wn8")
    wz_d = nc.inline_tensor(
        np.zeros((P, P), dtype=ml_dtypes.float8_e4m3fn), name="wz")

    HALF = HW // 2  # one PSUM bank of fp32

    # time blocks: [0], [1], [2,3], ..., [12,13], [14], [15]; single-step
    # blocks at the edges shorten the startup ramp and the drain tail
    blocks = ([[0], [1]] + [[t, t + 1] for t in range(2, T - 3, 2)]
              + [[T - 2], [T - 1]])

    with tile.TileContext(nc) as tc:
        with (
            tc.tile_pool(name="mp", bufs=2) as m_pool,
            tc.tile_pool(name="xp", bufs=3) as x_pool,
            tc.tile_pool(name="sp", bufs=4) as s_pool,
            tc.tile_pool(name="wp", bufs=1) as w_pool,
            tc.tile_pool(name="zp", bufs=1, space="PSUM") as z_pool,
        ):
            wn8 = w_pool.tile([P, P], FP8, tag="wn8", name="wn8")
            wz = w_pool.tile([P, P], FP8, tag="wz", name="wz")
            nc.sync.dma_start(wn8[:], wn8_d.ap()[:])
            nc.sync.dma_start(wz[:], wz_d.ap()[:])

            z_tiles = [
                z_pool.tile([P, HW], F32, tag=f"z{g}", name=f"z{g}")
                for g in range(NG)
            ]
            def load_block(blk, fanout=False):
                # [P, NG, len(blk)*HW]; the (t f) merge keeps the DMA AP 3-dim
                # and makes DRAM lines 8KB-contiguous when len(blk) == 2
                tb = len(blk)
                xt = x_pool.tile([P, NG, tb * HW], F32, tag="xt", name="xt")
                src = x_v[:, :, blk[0]:blk[0] + tb, :].rearrange(
                    "g p t f -> p g (t f)")
                if fanout:
                    # halves on the two HWDGE queues: independent completion
                    # counters (no ordering race), first groups land sooner
                    nc.sync.dma_start(xt[:, 0:2, :], src[:, 0:2, :])
                    nc.scalar.dma_start(xt[:, 2:4, :], src[:, 2:4, :])
                else:
                    nc.sync.dma_start(xt[:], src)
                return xt

            def store_block(blk, st, split=False):
                tb = len(blk)
                dst = s_v[:, :, blk[0]:blk[0] + tb, :].rearrange(
                    "g p t f -> p g (t f)")
                if split:
                    # final blocks: halve the drain tail by storing on both
                    # HWDGE queues (no later load triggers behind these, so
                    # the sync-queue wait cannot stall the pipeline)
                    nc.sync.dma_start(dst[:, 0:2, :], st[:, 0:2, :])
                    nc.scalar.dma_start(dst[:, 2:4, :], st[:, 2:4, :])
                else:
                    nc.scalar.dma_start(dst, st[:])

            # two blocks of lookahead in flight (x_pool bufs=3; the pool's
            # buffer-reuse semaphores gate deeper prefetch automatically)
            x_tiles = [load_block(blocks[0], fanout=True),
                       load_block(blocks[1])]
            for bi, blk in enumerate(blocks):
                if bi + 2 < len(blocks):
                    x_tiles.append(load_block(blocks[bi + 2]))
                x_blk = x_tiles[bi]
                st = s_pool.tile([P, NG, len(blk) * HW], FP8, tag="st",
                                 name="st")
                for ti, t in enumerate(blk):
                    tf = slice(ti * HW, (ti + 1) * HW)
                    srcs = []
                    for g in range(NG):
                        if t == 0:
                            srcs.append(x_blk[:, g, tf])
                        else:
                            mt = m_pool.tile([P, HW], F32, tag=f"m{g}",
                                             name=f"m{g}")
                            nc.vector.scalar_tensor_tensor(
                                mt[:], z_tiles[g][:], BETA,
                                x_blk[:, g, tf],
                                op0=ALU.mult, op1=ALU.add,
                            )
                            srcs.append(mt[:])
                    for g in range(NG):
                        nc.vector.tensor_scalar(
                            st[:, g, tf], srcs[g], THRESH, None, ALU.is_gt)
                    if t < T - 1:
                        for g in range(NG):
                            if t == 0:
                                # A start=False matmul on a bank whose
                                # per-element has_written bits are clear
                                # OVERWRITES instead of accumulating. Set the
                                # bits with a 0-weight pass before the copy.
                                for c in range(2):
                                    cols = slice(ti * HW + c * HALF,
                                                 ti * HW + (c + 1) * HALF)
                                    nc.tensor.matmul(
                                        z_tiles[g][:, c * HALF:(c + 1) * HALF],
                                        wz[:], st[:, g, cols],
                                        start=True, stop=False,
                                        skip_group_check=True,
                                    )
                            nc.scalar.copy(z_tiles[g][:], srcs[g])
                            for c in range(2):
                                cols = slice(ti * HW + c * HALF,
                                             ti * HW + (c + 1) * HALF)
                                nc.tensor.matmul(
                                    z_tiles[g][:, c * HALF:(c + 1) * HALF],
                                    wn8[:], st[:, g, cols],
                                    start=False, stop=True,
                                    skip_group_check=True,
                                )
                store_block(blk, st, split=(bi >= len(blocks) - 2))
    nc.compile()
    return nc


def _get_nc():
    global _nc_cache
    if _nc_cache is None:
        _nc_cache = _build()
    return _nc_cache


def _run(x, **spmd_kwargs):
    x = np.ascontiguousarray(np.asarray(x, dtype=np.float32))
    assert x.shape == (B, C, T, H, W)
    nc = _get_nc()
    in_maps = [
        {"x": x[i * B_PER:(i + 1) * B_PER].reshape(ROWS, T, HW)}
        for i in range(NCORES)
    ]
    res = run_bass_kernel_spmd(nc, in_maps, list(range(NCORES)), **spmd_kwargs)
    out = np.concatenate(
        [
            np.asarray(r["s"]).astype(np.float32).reshape(B_PER, C, T, H, W)
            for r in res.results
        ],
        axis=0,
    )
    return out, res


def kernel(x):
    out, _ = _run(x)
    return out


# revision 44
# speedup vs baseline: 1.1072x; 1.1072x over previous
"""LIF spiking-neuron (soft reset) Bass kernel for Trainium2, 8-core SPMD.

Input  x: [B=32, C=128, T=16, H=32, W=32] f32
Output s: same shape, spikes in {0, 1}.

Recurrence per element over T:
    m' = z * 0.75 + x_t              (integrate; z = post-reset membrane)
    s_t = (m' > 0.5)                 (spike)
    z   = m' - 0.5 * s_t             (soft reset)

Engine assignment (the point of this design): on TRN2, DVE and GpSimd
arbitrate an exclusive shared SBUF port pair, so GpSimd buys nothing while
DVE runs 2-src ops back-to-back; and every op here except the spike compare
is unavailable or slow elsewhere. The schedule keeps only two ops per
element on the DVE and farms the reset out to the private-port engines:

    DVE : m'_g(SBUF) = stt(z_g(PSUM) * beta + x_t,g)         ~1.2 us
    DVE : s_g(SBUF)  = (m'_g > 0.5) -> fp8e4 {0,1}           ~0.7 us
    ACT : copy m'_g -> z_g (PSUM overwrite; private ports)   ~1.1 us
    PE  : z_g += (-0.5 I_fp8) @ s_g  (matmul accumulate onto the ACT-written
          value; start=False adds unconditionally once the bank's per-element
          has_written bits are set - the t=0 zero-weight pass does that)
    DMA : s shipped as raw fp8 bytes (1B/elem); host maps to f32 {0,1}

All f32 roundings match the reference order bit-for-bit (the fp8 matmul
products are 1.0*-0.5 or 0.0, and m' - 0.5*s is exactly representable), so
rel err is 0. The spike tile doubles as the PE operand and the DMA output.

x loads and s stores are coalesced two timesteps per DMA: the DRAM layout
[row, t, f] makes a 2-step slice 8KB-contiguous per row, so transfers are
half as many with double the line length. The first load covers only t=0 to
keep the startup ramp short.

Sharding: batch dim split across 8 cores (4 per core); per core the shard is
[512 (b*c) rows, 16 t, 1024 hw], rows mapped to SBUF partitions in 4 groups
of 128. z lives in PSUM: 4 groups x 4KB = all 8 banks.
"""

import numpy as np

import concourse.bacc as bacc
import concourse.mybir as mybir
import concourse.tile as tile
from concourse.bass_utils import run_bass_kernel_spmd

B, C, T, H, W = 32, 128, 16, 32, 32
NCORES = 8
B_PER = B // NCORES          # 4
ROWS = B_PER * C             # 512
HW = H * W                   # 1024
P = 128
NG = ROWS // P               # 4 partition groups
BETA = 0.75
THRESH = 0.5

F32 = mybir.dt.float32
FP8 = mybir.dt.float8e4
ALU = mybir.AluOpType

_nc_cache = None


def _build():
    import ml_dtypes

    nc = bacc.Bacc(
        "TRN2",
        target_bir_lowering=False,
        debug=False,
        enable_asserts=False,
        num_devices=NCORES,
    )
    x_d = nc.dram_tensor("x", [ROWS, T, HW], F32, kind="ExternalInput").ap()
    s_d = nc.dram_tensor("s", [ROWS, T, HW], FP8, kind="ExternalOutput").ap()

    x_v = x_d.rearrange("(g p) t f -> g p t f", p=P)
    s_v = s_d.rearrange("(g p) t f -> g p t f", p=P)

    wn8_d = nc.inline_tensor(
        (-THRESH * np.eye(P)).astype(ml_dtypes.float8_e4m3fn), name="wn8")
    wz_d = nc.inline_tensor(
        np.zeros((P, P), dtype=ml_dtypes.float8_e4m3fn), name="wz")

    HALF = HW // 2  # one PSUM bank of fp32

    # time blocks: [0], [1], [2,3], ..., [12,13], [14], [15]; single-step
    # blocks at the edges shorten the startup ramp and the drain tail
    blocks = ([[0], [1]] + [[t, t + 1] for t in range(2, T - 3, 2)]
              + [[T - 2], [T - 1]])

    with tile.TileContext(nc) as tc:
        with (
            tc.tile_pool(name="mp", bufs=2) as m_pool,
            tc.tile_pool(name="xp", bufs=3) as x_pool,
            tc.tile_pool(name="sp", bufs=4) as s_pool,
            tc.tile_pool(name="wp", bufs=1) as w_pool,
            tc.tile_pool(name="zp", bufs=1, space="PSUM") as z_pool,
        ):
            wn8 = w_pool.tile([P, P], FP8, tag="wn8", name="wn8")
            wz = w_pool.tile([P, P], FP8, tag="wz", name="wz")
            nc.sync.dma_start(wn8[:], wn8_d.ap()[:])
            nc.sync.dma_start(wz[:], wz_d.ap()[:])

            z_tiles = [
                z_pool.tile([P, HW], F32, tag=f"z{g}", name=f"z{g}")
                for g in range(NG)
            ]
            def load_block(blk, fanout=False):
                # [P, NG, len(blk)*HW]; the (t f) merge keeps the DMA AP 3-dim
                # and makes DRAM lines 8KB-contiguous when len(blk) == 2
                tb = len(blk)
                xt = x_pool.tile([P, NG, tb * HW], F32, tag="xt", name="xt")
                src = x_v[:, :, blk[0]:blk[0] + tb, :].rearrange(
                    "g p t f -> p g (t f)")
                if fanout:
                    # halves on the two HWDGE queues: independent completion
                    # counters (no ordering race), first groups land sooner
                    nc.sync.dma_start(xt[:, 0:2, :], src[:, 0:2, :])
                    nc.scalar.dma_start(xt[:, 2:4, :], src[:, 2:4, :])
                else:
                    nc.sync.dma_start(xt[:], src)
                return xt

            def store_block(blk, st):
                tb = len(blk)
                nc.scalar.dma_start(
                    s_v[:, :, blk[0]:blk[0] + tb, :].rearrange(
                        "g p t f -> p g (t f)"),
                    st[:],
                )

            # two blocks of lookahead in flight (x_pool bufs=3; the pool's
            # buffer-reuse semaphores gate deeper prefetch automatically)
            x_tiles = [load_block(blocks[0], fanout=True),
                       load_block(blocks[1])]
            for bi, blk in enumerate(blocks):
                if bi + 2 < len(blocks):
                    x_tiles.append(load_block(blocks[bi + 2]))
                x_blk = x_tiles[bi]
                st = s_pool.tile([P, NG, len(blk) * HW], FP8, tag="st",
                                 name="st")
                for ti, t in enumerate(blk):
                    tf = slice(ti * HW, (ti + 1) * HW)
                    srcs = []
                    for g in range(NG):
                        if t == 0:
                            srcs.append(x_blk[:, g, tf])
                        else:
                            mt = m_pool.tile([P, HW], F32, tag=f"m{g}",
                                             name=f"m{g}")
                            nc.vector.scalar_tensor_tensor(
                                mt[:], z_tiles[g][:], BETA,
                                x_blk[:, g, tf],
                                op0=ALU.mult, op1=ALU.add,
                            )
                            srcs.append(mt[:])
                    for g in range(NG):
                        nc.vector.tensor_scalar(
                            st[:, g, tf], srcs[g], THRESH, None, ALU.is_gt)
                    if t < T - 1:
                        for g in range(NG):
                            if t == 0:
                                # A start=False matmul on a bank whose
                                # per-element has_written bits are clear
                                # OVERWRITES instead of accumulating. Set the
                                # bits with a 0-weight pass before the copy.
                                for c in range(2):
                                    cols = slice(ti * HW + c * HALF,
                                                 ti * HW + (c + 1) * HALF)
                                    nc.tensor.matmul(
                                        z_tiles[g][:, c * HALF:(c + 1) * HALF],
                                        wz[:], st[:, g, cols],
                                        start=True, stop=False,
                                        skip_group_check=True,
                                    )
                            nc.scalar.copy(z_tiles[g][:], srcs[g])
                            for c in range(2):
                                cols = slice(ti * HW + c * HALF,
                                             ti * HW + (c + 1) * HALF)
                                nc.tensor.matmul(
                                    z_tiles[g][:, c * HALF:(c + 1) * HALF],
                                    wn8[:], st[:, g, cols],
                                    start=False, stop=True,
                                    skip_group_check=True,
                                )
                store_block(blk, st)
    nc.compile()
    return nc


def _get_nc():
    global _nc_cache
    if _nc_cache is None:
        _nc_cache = _build()
    return _nc_cache


def _run(x, **spmd_kwargs):
    x = np.ascontiguousarray(np.asarray(x, dtype=np.float32))
    assert x.shape == (B, C, T, H, W)
    nc = _get_nc()
    in_maps = [
        {"x": x[i * B_PER:(i + 1) * B_PER].reshape(ROWS, T, HW)}
        for i in range(NCORES)
    ]
    res = run_bass_kernel_spmd(nc, in_maps, list(range(NCORES)), **spmd_kwargs)
    out = np.concatenate(
        [
            np.asarray(r["s"]).astype(np.float32).reshape(B_PER, C, T, H, W)
            for r in res.results
        ],
        axis=0,
    )
    return out, res


def kernel(x):
    out, _ = _run(x)
    return out
